# revision 17
# baseline (speedup 1.0000x reference)
# Cross-attention kernel for Trainium2, 8 NeuronCores.
#
# Reference computation (per batch b):
#   Q = q @ Wq.T + bq ; K = k @ Wk.T + bk ; V = v @ Wv.T + bv      [N, D]
#   per head h (D=1024, H=16, hd=64):
#     S = Qh @ Kh.T * D**-0.5 ; P = softmax(S, axis=-1) ; O = P @ Vh
#   out = concat_h(O) @ Wo.T + bo
#
# Sharding: 8 cores = 4 batches x 2 head-groups (8 heads / 512 channels each).
# Each core computes its batch's projections restricted to its 512 channels,
# attention for its 8 heads, and a partial output projection; the host sums
# the two partials per batch and adds bo.
#
# Device layout (all activations transposed so no on-device transposes occur):
#   qT/kT/vT  [D, N]   (host-transposed, bf16)
#   QT'/KT'   [c, n]   channels on partitions -> heads are partition ranges
#   S^T       [m, n]   keys on partitions -> PV consumes exp(S^T) directly
#   rowsum    via [V | ones] augmented PV stationary (M=65), free.
#   softmax   has no max-subtraction: |S| < ~1 for this problem by
#             construction (verified numerically on the host side).
#
# QK^T runs in 64x128 row-tiled PE mode: head A (SBUF partitions 0-63) and
# head B (64-127) stream concurrently on the two 64-row PE tiles into
# different PSUM banks of one [128, 2*NBS] S tile, so contraction K=64 still
# uses the full array and exp gets a single [128, 2*NBS] activation call.

import numpy as np
import ml_dtypes
from contextlib import ExitStack

import concourse.bacc as bacc
import concourse.bass as bass
import concourse.mybir as mybir
import concourse.tile as tile
from concourse.bass_utils import run_bass_kernel_spmd

F32 = mybir.dt.float32
BF16 = mybir.dt.bfloat16
AluOp = mybir.AluOpType
Act = mybir.ActivationFunctionType

# full-problem constants
B, N_FULL, M_FULL, D_FULL = 4, 2048, 2048, 1024
HEADS, HD = 16, 64
N_CORES = 8
GROUPS = N_CORES // B  # head groups per batch (2)


def build_program(N, M, D, DH, HD, nbs=512, trn_type="TRN2"):
    """Build the per-core Bass program.

    N: query rows, M: key rows, D: model/contraction dim,
    DH: per-core channels (this group's heads * HD), HD: head dim,
    nbs: query-block size (free dim of S^T tiles).
    """
    P = 128
    H = DH // HD          # local heads
    HP = H // 2           # head pairs == channel chunks
    KC = D // P           # contraction chunks
    CC = DH // P          # channel chunks (== HP)
    MC = M // P           # key chunks
    NB = N // nbs         # query blocks
    EB = max(D // 512, 1) # output-proj column blocks
    EBS = min(D, 512)
    scale = float(D) ** -0.5
    assert CC == HP and H % 2 == 0 and M % P == 0 and N % nbs == 0

    nc = bacc.Bacc(trn_type, target_bir_lowering=False, debug=False,
                   enable_asserts=False, num_devices=1)

    qT = nc.dram_tensor("qT", [D, N], BF16, kind="ExternalInput")
    kT = nc.dram_tensor("kT", [D, M], BF16, kind="ExternalInput")
    vT = nc.dram_tensor("vT", [D, M], BF16, kind="ExternalInput")
    wqT = nc.dram_tensor("wqT", [D, DH], BF16, kind="ExternalInput")
    wkT = nc.dram_tensor("wkT", [D, DH], BF16, kind="ExternalInput")
    wvT = nc.dram_tensor("wvT", [D, DH], BF16, kind="ExternalInput")
    woT = nc.dram_tensor("woT", [DH, D], BF16, kind="ExternalInput")
    bq = nc.dram_tensor("bq", [P, CC], F32, kind="ExternalInput")
    bk = nc.dram_tensor("bk", [P, CC], F32, kind="ExternalInput")
    bv = nc.dram_tensor("bv", [1, DH], F32, kind="ExternalInput")
    out = nc.dram_tensor("out", [N, D], F32, kind="ExternalOutput")

    with tile.TileContext(nc) as tc, ExitStack() as ctx:
        const = ctx.enter_context(tc.tile_pool(name="const", bufs=1))
        wpool = ctx.enter_context(tc.tile_pool(name="wpool", bufs=1))
        persist = ctx.enter_context(tc.tile_pool(name="persist", bufs=1))
        small = ctx.enter_context(tc.tile_pool(name="small", bufs=2))
        ob_pool = ctx.enter_context(tc.tile_pool(name="ob_pool", bufs=2))
        qkv_pool = ctx.enter_context(tc.tile_pool(name="qkv_pool",
                                                  bufs=2 * KC + 2))
        v_pool = ctx.enter_context(tc.tile_pool(name="v_pool", bufs=KC + 1))
        qtkt = ctx.enter_context(tc.tile_pool(name="qtkt", bufs=2))
        e_pool = ctx.enter_context(tc.tile_pool(name="e_pool", bufs=MC + 2))
        # one PSUM pool: tag "s" = 2 x [P, 2*nbs] (2 banks each), shared by
        # S^T tiles, Q/K-proj accumulators (bank halves) and out-proj;
        # tag "o" = 4 x 1 bank, shared by PV accumulators and V-proj.
        psum = ctx.enter_context(tc.tile_pool(name="psum", bufs=2,
                                              space="PSUM"))

        # ---- constants / weights ----
        bq_sb = const.tile([P, CC], F32)
        nc.sync.dma_start(bq_sb, bq.ap())
        bk_sb = const.tile([P, CC], F32)
        nc.sync.dma_start(bk_sb, bk.ap())
        bv_row = const.tile([1, DH], F32)
        nc.sync.dma_start(bv_row, bv.ap())
        bv_bc = const.tile([P, DH], F32)
        nc.gpsimd.partition_broadcast(bv_bc, bv_row)

        wq_sb = wpool.tile([P, KC, DH], BF16)
        nc.sync.dma_start(wq_sb, wqT.ap().rearrange("(kc p) c -> p kc c", p=P))
        wk_sb = wpool.tile([P, KC, DH], BF16)
        nc.sync.dma_start(wk_sb, wkT.ap().rearrange("(kc p) c -> p kc c", p=P))
        # wv dies after V-proj; wo loads late into the same slot
        wv_sb = wpool.tile([P, KC * DH], BF16, name="wv_sb", tag="w2")
        nc.sync.dma_start(
            wv_sb.rearrange("p (kc c) -> p kc c", c=DH),
            wvT.ap().rearrange("(kc p) c -> p kc c", p=P))
        wv_v = wv_sb.rearrange("p (kc c) -> p kc c", c=DH)

        # V' with a ones column appended per head: [m, H*(HD+1)]
        vpp = persist.tile([P, MC, H * (HD + 1)], BF16)
        ont = persist.tile([P, CC, N], BF16)     # normalized O^T
        vpp_v = vpp.rearrange("p mc (h c) -> p mc h c", c=HD + 1)

        # v in half-m chunks (separate small pool; q/k stream per head-pair)
        v_r = vT.ap().rearrange("(kc p) (h m) -> h kc p m", p=P, h=2)
        MCH = MC // 2

        def v_proj(half):
            vch = []
            for kc in range(KC):
                ch = v_pool.tile([P, M // 2], BF16, name=f"v{half}_{kc}",
                                 tag="v")
                nc.sync.dma_start(ch, v_r[half, kc])
                vch.append(ch)
            for mb in range(half * MCH, (half + 1) * MCH):
                ps = psum.tile([P, DH], F32, name=f"vp{mb}", tag="o", bufs=4)
                lo = (mb - half * MCH) * P
                for kc in range(KC):
                    nc.tensor.matmul(
                        ps, lhsT=vch[kc][:, lo:lo + P], rhs=wv_v[:, kc, :],
                        start=(kc == 0), stop=(kc == KC - 1))
                nc.vector.tensor_tensor(
                    out=vpp_v[:, mb, :, 0:HD],
                    in0=ps.rearrange("p (h c) -> p h c", c=HD),
                    in1=bv_bc.rearrange("p (h c) -> p h c", c=HD),
                    op=AluOp.add)

        # q/k inputs stream as column-half chunk sets; each proj "block"
        # projects one query-block of Q AND K into the two bank-halves of a
        # single s-slot, so staged projections never starve the exp ring.
        assert N == M
        q_r = qT.ap().rearrange("(kc p) (h n) -> h kc p n", p=P, h=2)
        k_r = kT.ap().rearrange("(kc p) (h n) -> h kc p n", p=P, h=2)
        NBH = max(NB // 2, 1)  # query blocks per column-half

        def load_half(src_r, hp, half, pfx):
            chs = []
            for kc in range(KC):
                ch = qkv_pool.tile([P, N // 2], BF16,
                                   name=f"{pfx}{hp}_{half}_{kc}", tag="qkv")
                nc.sync.dma_start(ch, src_r[half, kc])
                chs.append(ch)
            return chs

        def proj_block(hp, qch, kch, qdst, kdst, i):
            ps = psum.tile([P, 2 * nbs], F32, name=f"pb{hp}_{i}", tag="s",
                           bufs=2)
            lo = (i % NBH) * nbs
            for kc in range(KC):
                nc.tensor.matmul(
                    ps[:, 0:nbs], lhsT=wq_sb[:, kc, hp * P:(hp + 1) * P],
                    rhs=qch[kc][:, lo:lo + nbs],
                    start=(kc == 0), stop=(kc == KC - 1))
                nc.tensor.matmul(
                    ps[:, nbs:2 * nbs],
                    lhsT=wk_sb[:, kc, hp * P:(hp + 1) * P],
                    rhs=kch[kc][:, lo:lo + nbs],
                    start=(kc == 0), stop=(kc == KC - 1))
            nc.vector.tensor_scalar(
                out=qdst[:, i * nbs:(i + 1) * nbs], in0=ps[:, 0:nbs],
                scalar1=bq_sb[:, hp:hp + 1], scalar2=None, op0=AluOp.add)
            nc.vector.tensor_scalar(
                out=kdst[:, i * nbs:(i + 1) * nbs], in0=ps[:, nbs:2 * nbs],
                scalar1=bk_sb[:, hp:hp + 1], scalar2=None, op0=AluOp.add)

        def new_qtkt(hp):
            qt_n = qtkt.tile([P, N], BF16, name=f"qt{hp}", tag="qt")
            kt_n = qtkt.tile([P, M], BF16, name=f"kt{hp}", tag="kt")
            return qt_n, kt_n

        # ---- main loop: attention(hp) with proj(hp+1) staged inside ----
        qt_hp, kt_hp = new_qtkt(0)
        qch = load_half(q_r, 0, 0, "q")
        kch = load_half(k_r, 0, 0, "k")
        for i in range(NB):
            if i == NBH:
                qch = load_half(q_r, 0, 1, "q")
                kch = load_half(k_r, 0, 1, "k")
            proj_block(0, qch, kch, qt_hp, kt_hp, i)
        wo_sb = None
        qt_nxt = kt_nxt = qch_n = kch_n = None
        for hp in range(HP):
            hA, hB = 2 * hp, 2 * hp + 1
            if hp == 0:
                v_proj(0)
                v_proj(1)
                nc.vector.memset(vpp_v[:, :, :, HD:HD + 1], 1.0)
                # wo reuses wv's slot once V-proj is done with it
                wo_sb = wpool.tile([P, CC * D], BF16, name="wo_sb", tag="w2")
                nc.sync.dma_start(
                    wo_sb.rearrange("p (cc e) -> p cc e", e=D),
                    woT.ap().rearrange("(cc p) e -> p cc e", p=P))
            for b in range(NB):
                # stage the next head-pair's projection, one block per
                # attention block, so exp never loses the whole s-ring
                if hp + 1 < HP:
                    if b == 0:
                        qt_nxt, kt_nxt = new_qtkt(hp + 1)
                    if b % NBH == 0:
                        qch_n = load_half(q_r, hp + 1, b // NBH, "q")
                        kch_n = load_half(k_r, hp + 1, b // NBH, "k")
                    proj_block(hp + 1, qch_n, kch_n, qt_nxt, kt_nxt, b)
                nsl = slice(b * nbs, (b + 1) * nbs)
                # per head: even/odd m-half accumulators so PV also runs on
                # the two 64-row PE tiles concurrently (no PSUM bank sharing)
                oacc = [psum.tile([HD + 1, nbs], F32, name=f"o{i}", tag="o",
                                  bufs=4) for i in range(4)]
                e_tiles = []
                for mc in range(MC):
                    s = psum.tile([P, 2 * nbs], F32, name="s", tag="s",
                                  bufs=2)
                    # head A on PE rows 0-63, head B on rows 64-127
                    nc.tensor.matmul(
                        s[:, 0:nbs],
                        lhsT=kt_hp[0:64, mc * P:(mc + 1) * P],
                        rhs=qt_hp[0:64, nsl], start=True, stop=True)
                    nc.tensor.matmul(
                        s[:, nbs:2 * nbs],
                        lhsT=kt_hp[64:P, mc * P:(mc + 1) * P],
                        rhs=qt_hp[64:P, nsl], start=True, stop=True)
                    e = e_pool.tile([P, 2 * nbs], BF16, name="e", tag="e")
                    nc.scalar.activation(e, s, Act.Exp, scale=scale)
                    e_tiles.append(e)
                for mc in range(MC):
                    st, sp = mc == 0, mc == MC - 1
                    for h_i, h in ((0, hA), (1, hB)):
                        nc.tensor.matmul(
                            oacc[2 * h_i],
                            lhsT=vpp_v[0:64, mc, h, :],
                            rhs=e_tiles[mc][0:64, h_i * nbs:(h_i + 1) * nbs],
                            start=st, stop=sp)
                        nc.tensor.matmul(
                            oacc[2 * h_i + 1],
                            lhsT=vpp_v[64:P, mc, h, :],
                            rhs=e_tiles[mc][64:P, h_i * nbs:(h_i + 1) * nbs],
                            start=st, stop=sp)
                for h_i, lo in ((0, 0), (1, 64)):
                    tmp = small.tile([HD + 1, nbs], F32, name="tmp", tag="tmp")
                    # two PSUM reads in one DVE op are illegal; copy then add
                    nc.vector.tensor_copy(tmp, oacc[2 * h_i])
                    nc.vector.tensor_tensor(
                        out=tmp, in0=tmp, in1=oacc[2 * h_i + 1], op=AluOp.add)
                    rs = small.tile([1, nbs], F32, name="rs", tag="rs")
                    nc.vector.reciprocal(rs, tmp[HD:HD + 1, :])
                    bc = small.tile([64, nbs], F32, name="bc", tag="bc")
                    nc.gpsimd.partition_broadcast(bc, rs)
                    nc.vector.tensor_tensor(
                        out=ont[lo:lo + 64, hp, nsl], in0=tmp[0:HD, :],
                        in1=bc, op=AluOp.mult)
            qt_hp, kt_hp = qt_nxt, kt_nxt

        # ---- output projection: out[n, e] = sum_c O^T[c, n] * WoT[c, e] ----
        wo_v = wo_sb.rearrange("p (cc e) -> p cc e", e=D)
        assert EB * EBS <= 2 * nbs
        for ncs in range(N // P):
            po = psum.tile([P, 2 * nbs], F32, name=f"po{ncs}", tag="s",
                           bufs=2)
            pse = [po[:, eb * EBS:(eb + 1) * EBS] for eb in range(EB)]
            for cc in range(CC):
                for eb in range(EB):
                    nc.tensor.matmul(
                        pse[eb], lhsT=ont[:, cc, ncs * P:(ncs + 1) * P],
                        rhs=wo_v[:, cc, eb * EBS:(eb + 1) * EBS],
                        start=(cc == 0), stop=(cc == CC - 1))
            for eb in range(EB):
                ob = ob_pool.tile([P, EBS], F32, name="ob", tag="ob")
                nc.vector.tensor_copy(ob, pse[eb])
                nc.sync.dma_start(
                    out.ap()[ncs * P:(ncs + 1) * P, eb * EBS:(eb + 1) * EBS],
                    ob)

    nc.compile()
    return nc


_PROGRAM = None


def _get_program():
    global _PROGRAM
    if _PROGRAM is None:
        _PROGRAM = build_program(N_FULL, M_FULL, D_FULL,
                                 D_FULL // GROUPS, HD)
    return _PROGRAM


def _prep_inputs(q, k, v, Wq, bq, Wk, bk, Wv, bv, Wo, bo):
    """Host-side shard + layout prep -> per-core input dicts."""
    bf = ml_dtypes.bfloat16
    DH = D_FULL // GROUPS
    CC = DH // 128
    f32 = np.float32

    qT = [np.ascontiguousarray(np.asarray(q[b], f32).T).astype(bf)
          for b in range(B)]
    kTb = [np.ascontiguousarray(np.asarray(k[b], f32).T).astype(bf)
           for b in range(B)]
    vTb = [np.ascontiguousarray(np.asarray(v[b], f32).T).astype(bf)
           for b in range(B)]
    WqT = np.asarray(Wq, f32).T
    WkT = np.asarray(Wk, f32).T
    WvT = np.asarray(Wv, f32).T
    WoT = np.asarray(Wo, f32).T
    bq = np.asarray(bq, f32); bk = np.asarray(bk, f32)
    bv = np.asarray(bv, f32)

    per_g = []
    for g in range(GROUPS):
        cs = slice(g * DH, (g + 1) * DH)
        per_g.append({
            "wqT": np.ascontiguousarray(WqT[:, cs]).astype(bf),
            "wkT": np.ascontiguousarray(WkT[:, cs]).astype(bf),
            "wvT": np.ascontiguousarray(WvT[:, cs]).astype(bf),
            "woT": np.ascontiguousarray(WoT[cs, :]).astype(bf),
            "bq": np.ascontiguousarray(bq[cs].reshape(CC, 128).T),
            "bk": np.ascontiguousarray(bk[cs].reshape(CC, 128).T),
            "bv": np.ascontiguousarray(bv[cs].reshape(1, DH)),
        })

    in_maps = []
    for b in range(B):
        for g in range(GROUPS):
            m = {"qT": qT[b], "kT": kTb[b], "vT": vTb[b]}
            m.update(per_g[g])
            in_maps.append(m)
    return in_maps


LAST_RESULT = None


def kernel(q, k, v, Wq, bq, Wk, bk, Wv, bv, Wo, bo):
    global LAST_RESULT
    nc = _get_program()
    in_maps = _prep_inputs(q, k, v, Wq, bq, Wk, bk, Wv, bv, Wo, bo)
    res = run_bass_kernel_spmd(nc, in_maps, core_ids=list(range(N_CORES)))
    LAST_RESULT = res
    bo = np.asarray(bo, np.float32)
    outs = [res.results[b * GROUPS]["out"] + res.results[b * GROUPS + 1]["out"]
            + bo for b in range(B)]
    return np.stack(outs).astype(np.float32)
